# revision 2
# baseline (speedup 1.0000x reference)
"""MinGRU cell on 8 Trainium2 NeuronCores.

Math: per (batch b, hidden channel j) the reference's log-space scan equals
the linear recurrence

    h_t = c_t * h_{t-1} + v_t,      h_0 = g(h0)
    c_t = 1 - sigmoid(kz_t) = sigmoid(-kz_t)
    v_t = sigmoid(kz_t) * g(kh_t)
    kz = x @ Wz^T + bz,  kh = x @ Wh^T + bh
    g(u) = max(sigmoid(u), u + 0.5)          (exact identity)

All quantities are positive and O(1) (h_t is a convex combination), so the
linear-space recurrence is numerically fine (~2.3e-3 max rel err vs the
log-space reference, tolerance 2e-2).

Sharding: data-parallel over batch, one row per core (B == 8 cores), weights
replicated. Per core the tensor engine produces kz/kh in [h-partition,
s-free] layout, accumulating 8x 128-deep contractions per PSUM tile; the
recurrence runs as one tensor_tensor_scan per (h-tile, s-block), chained via
the previous block's last column.

The kernel is tensor-engine-bound: the two GEMMs are 8.6 GMAC/core =
524288 PE rows; at 1 row/cycle (fp32r/fp16/bf16 alike) and 2.4 GHz that is
~219 us, measured steady-state ~220-228 us. Design choices serve that
bound:

  - fp16 matmul operands (x, Wz, Wh): same PE speed as fp32r but half the
    DMA bytes and SBUF write traffic; measured max rel err 2.1e-3 for the
    GEMM quantization (fp8 would halve PE time via DoubleRow but measured
    0.26 rel err -- far outside tolerance; bf16 measured 1.8e-2 -- too close
    to the 2e-2 gate).
  - fp16 output: h rel err budget allows it; halves output DMA.
  - PSUM pools 4+4 (all 8 banks) so the PE never waits on consumers.
  - Prologue: weights stream in per-h-tile 0.5MB chunks with the first x
    block interleaved after the hi=0 chunks, so the first matmul starts
    after ~2MB of DMA instead of 9MB.
  - Output stores split in halves so the final store overlaps the last
    h-tiles' scans (shorter drain).

Elementwise work is balanced across the two fast pointwise engines
(~3 ops/tile each, all comfortably under the PE bound; gpsimd measured
slower and is left idle):

  ACT : z = sigmoid(kz+bz), a = sigmoid(kh+bh), c = sigmoid(-(kz+bz))
  DVE : t = (kh+bh+0.5) max a   [one fused scalar_tensor_tensor],
        v = z*t, scan

Host-side layout only (no math): x is fed pre-transposed (D, S) per batch so
the contraction dim lands on partitions; output comes back (H, S) fp16 and
is transposed/upcast on the host.
"""

import numpy as np

import concourse.bass as bass
import concourse.mybir as mybir
import concourse.tile as tile
from concourse import bacc
from concourse.bass_utils import run_bass_kernel_spmd

B, S, D, H = 8, 4096, 1024, 1024
N_CORES = 8
P = 128
SB = 512
NSB = S // SB
DT = D // P
HT = H // P

F32 = mybir.dt.float32
MM_DT = mybir.dt.float16
OUT_DT = mybir.dt.float16

_CACHE = {}


def _build_program(ablate=(), repeat=1, mm_dt=None, bufs=None,
                   out_dt=None, split_out=True):
    if mm_dt is None:
        mm_dt = MM_DT
    if out_dt is None:
        out_dt = OUT_DT
    bufs = {**{"xin": 3, "psz": 4, "psh": 4, "inter": 3, "outp": 2},
            **(bufs or {})}
    nc = bacc.Bacc(trn_type="TRN2")

    xT = nc.dram_tensor("xt", [D, S], mm_dt, kind="ExternalInput")
    wzT = nc.dram_tensor("wzt", [D, H], mm_dt, kind="ExternalInput")
    whT = nc.dram_tensor("wht", [D, H], mm_dt, kind="ExternalInput")
    bzg = nc.dram_tensor("bzg", [P, HT], F32, kind="ExternalInput")
    bhg = nc.dram_tensor("bhg", [P, HT], F32, kind="ExternalInput")
    h0g = nc.dram_tensor("h0g", [P, HT], F32, kind="ExternalInput")
    hT = nc.dram_tensor("ht", [H, S], out_dt, kind="ExternalOutput")

    AF = mybir.ActivationFunctionType
    OP = mybir.AluOpType

    with tile.TileContext(nc) as tc:
        with (
            tc.tile_pool(name="wpool", bufs=1) as wpool,
            tc.tile_pool(name="bias", bufs=1) as bias,
            tc.tile_pool(name="xin", bufs=bufs["xin"]) as xin,
            tc.tile_pool(name="psz", bufs=bufs["psz"], space="PSUM") as psz,
            tc.tile_pool(name="psh", bufs=bufs["psh"], space="PSUM") as psh,
            tc.tile_pool(name="inter", bufs=bufs["inter"]) as inter,
            tc.tile_pool(name="outp", bufs=bufs["outp"]) as outp,
        ):
            xT_v0 = xT.ap().rearrange("(dt p) s -> p dt s", p=P)
            wz_sb = wpool.tile([P, DT, H], mm_dt, tag="wz")
            wh_sb = wpool.tile([P, DT, H], mm_dt, tag="wh")
            wzT_v = wzT.ap().rearrange("(dt p) h -> p dt h", p=P)
            whT_v = whT.ap().rearrange("(dt p) h -> p dt h", p=P)
            x_first = xin.tile([P, DT, SB], mm_dt, tag="x")

            # Prologue order: weights for the first h-tile, then the first
            # x block, then the remaining per-h-tile weight chunks. The
            # hi=0 matmuls start after ~2MB of DMA instead of 9MB, and the
            # later chunks stream in while the PE computes.
            def w_chunk(hi):
                hsl = slice(hi * P, (hi + 1) * P)
                nc.sync.dma_start(out=wz_sb[:, :, hsl], in_=wzT_v[:, :, hsl])
                nc.sync.dma_start(out=wh_sb[:, :, hsl], in_=whT_v[:, :, hsl])

            w_chunk(0)
            if "xdma" not in ablate:
                nc.sync.dma_start(out=x_first, in_=xT_v0[:, :, 0:SB])
            for hi in range(1, HT):
                w_chunk(hi)

            bz_sb = bias.tile([P, HT], F32, tag="bz")
            nc.sync.dma_start(out=bz_sb, in_=bzg.ap())
            bh_sb = bias.tile([P, HT], F32, tag="bh")
            nc.sync.dma_start(out=bh_sb, in_=bhg.ap())
            h0_sb = bias.tile([P, HT], F32, tag="h0")
            nc.sync.dma_start(out=h0_sb, in_=h0g.ap())

            bhh_sb = bias.tile([P, HT], F32, tag="bhh")
            nc.vector.tensor_scalar_add(bhh_sb[:], bh_sb[:], 0.5)
            nbz_sb = bias.tile([P, HT], F32, tag="nbz")
            nc.vector.tensor_scalar_mul(nbz_sb[:], bz_sb[:], -1.0)

            g0_s = bias.tile([P, HT], F32, tag="g0s")
            nc.scalar.activation(g0_s[:], h0_sb[:], AF.Sigmoid)
            g0_t = bias.tile([P, HT], F32, tag="g0t")
            nc.vector.tensor_scalar_add(g0_t[:], h0_sb[:], 0.5)
            g0 = bias.tile([P, HT], F32, tag="g0")
            nc.vector.tensor_max(g0[:], g0_s[:], g0_t[:])

            xT_v = xT.ap().rearrange("(dt p) s -> p dt s", p=P)
            hT_v = hT.ap().rearrange("(ht p) s -> p ht s", p=P)

            for _rep in range(repeat):
              prev_out = None
              for sb in range(NSB):
                if _rep == 0 and sb == 0:
                    x_t = x_first
                else:
                    x_t = xin.tile([P, DT, SB], mm_dt, tag="x")
                    if "xdma" not in ablate:
                        nc.sync.dma_start(
                            out=x_t, in_=xT_v[:, :, sb * SB:(sb + 1) * SB]
                        )

                ot = outp.tile([P, HT, SB], out_dt, tag="o")
                for hi in range(HT):
                    kz = psz.tile([P, SB], F32)
                    kh = psh.tile([P, SB], F32)
                    if "mm" not in ablate:
                        for di in range(DT):
                            nc.tensor.matmul(
                                kz[:],
                                wz_sb[:, di, hi * P:(hi + 1) * P],
                                x_t[:, di, :],
                                start=(di == 0),
                                stop=(di == DT - 1),
                            )
                        for di in range(DT):
                            nc.tensor.matmul(
                                kh[:],
                                wh_sb[:, di, hi * P:(hi + 1) * P],
                                x_t[:, di, :],
                                start=(di == 0),
                                stop=(di == DT - 1),
                            )

                    bcol = slice(hi, hi + 1)
                    zt = inter.tile([P, SB], F32, tag="z")
                    at = inter.tile([P, SB], F32, tag="a")
                    tl = inter.tile([P, SB], F32, tag="tl")
                    ct = inter.tile([P, SB], F32, tag="c")
                    vt = inter.tile([P, SB], F32, tag="v")

                    if "act" not in ablate:
                        nc.scalar.activation(
                            zt[:], kz[:], AF.Sigmoid, bias=bz_sb[:, bcol]
                        )
                        nc.scalar.activation(
                            at[:], kh[:], AF.Sigmoid, bias=bh_sb[:, bcol]
                        )
                        # c = 1 - z = sigmoid(-(kz + bz))
                        nc.scalar.activation(
                            ct[:], kz[:], AF.Sigmoid,
                            bias=nbz_sb[:, bcol], scale=-1.0,
                        )

                    if "dve" not in ablate:
                        nc.vector.scalar_tensor_tensor(
                            tl[:], kh[:], bhh_sb[:, bcol], at[:],
                            op0=OP.add, op1=OP.max,
                        )
                        nc.vector.tensor_mul(vt[:], zt[:], tl[:])

                    if "scan" not in ablate:
                        init = (
                            g0[:, hi:hi + 1] if sb == 0
                            else prev_out[:, hi, SB - 1:SB]
                        )
                        nc.vector.tensor_tensor_scan(
                            ot[:, hi, :], ct[:], vt[:], init,
                            op0=OP.mult, op1=OP.add,
                        )
                if "scan" not in ablate:
                    prev_out = ot
                if "outdma" not in ablate:
                    ssl = slice(sb * SB, (sb + 1) * SB)
                    if split_out:
                        # store in halves: the first half goes out while the
                        # last h-tiles are still scanning
                        nc.sync.dma_start(
                            out=hT_v[:, 0:HT // 2, ssl],
                            in_=ot[:, 0:HT // 2, :])
                        nc.sync.dma_start(
                            out=hT_v[:, HT // 2:HT, ssl],
                            in_=ot[:, HT // 2:HT, :])
                    else:
                        nc.sync.dma_start(out=hT_v[:, :, ssl], in_=ot[:])
    nc.finalize()
    return nc


def _get_program():
    if "nc" not in _CACHE:
        _CACHE["nc"] = _build_program()
    return _CACHE["nc"]


def run(inputs, **kw):
    x = np.asarray(inputs["x"], dtype=np.float32)
    h0 = np.asarray(inputs["h0"], dtype=np.float32)
    Wz = np.asarray(inputs["Wz"], dtype=np.float32)
    bz = np.asarray(inputs["bz"], dtype=np.float32)
    Wh = np.asarray(inputs["Wh"], dtype=np.float32)
    bh = np.asarray(inputs["bh"], dtype=np.float32)

    mm_np = mybir.dt.np(MM_DT)
    wzT = np.ascontiguousarray(Wz.T).astype(mm_np)
    whT = np.ascontiguousarray(Wh.T).astype(mm_np)
    bzg = np.ascontiguousarray(bz.reshape(HT, P).T)
    bhg = np.ascontiguousarray(bh.reshape(HT, P).T)

    in_maps = []
    for b in range(N_CORES):
        in_maps.append({
            "xt": np.ascontiguousarray(x[b].T).astype(mm_np),
            "wzt": wzT,
            "wht": whT,
            "bzg": bzg,
            "bhg": bhg,
            "h0g": np.ascontiguousarray(h0[b, 0].reshape(HT, P).T),
        })

    nc = _get_program()
    res = run_bass_kernel_spmd(nc, in_maps, core_ids=list(range(N_CORES)), **kw)
    out = np.stack(
        [res.results[b]["ht"].T.astype(np.float32) for b in range(N_CORES)],
        axis=0)
    return np.ascontiguousarray(out), res


def kernel(**inputs):
    out, _ = run(inputs)
    return out
